# revision 13
# baseline (speedup 1.0000x reference)
"""Custom LSTM cell (H=4096) on 8 Trainium2 NeuronCores.

Tensor-parallel: each gate weight [H, 2H] is sharded row-wise (output dim)
across 8 cores; core i computes its h/c slice [512] with no cross-core
communication. Gather on host.

Per-core math (HS = 512 rows of each gate):
  f = sigmoid(W_f z_hx + b_f); i = sigmoid(W_i z_hx + b_i)
  Ct = tanh(W_C z_hc + b_C);   o = sigmoid(W_o z_hx + b_o)
  C = f*c + i*Ct;  h = o*tanh(C)
with z_hx = cat(h, x), z_hc = cat(h, c)  (faithful reference quirk).

Matvec on TensorE with z stationary ([128,1] column per k-tile) and the
weight slice moving ([128,512] f32r at full 1 cycle/row rate); PSUM
accumulates over 64 k-tiles per gate. Weights are pre-transposed and
pre-tiled on host into chunk-contiguous layout so every weight DMA is a
[128 part x 16KB contig] 2MB transfer.
"""

import numpy as np

import concourse.bacc as bacc
import concourse.bass as bass
import concourse.mybir as mybir
import concourse.tile as tile
from concourse.bass_utils import run_bass_kernel_spmd

H = 4096
NCORES = 8
HS = H // NCORES          # 512 output rows per core
K2 = 2 * H                # 8192 contraction dim
NKT = K2 // 128           # 64 k-tiles
CH = 8                    # k-tiles per weight DMA chunk (2 MB)
NCHUNK = NKT // CH        # 8 chunks
F32 = mybir.dt.float32
F32R = mybir.dt.float32r

# gate order: (name, uses z_hc?)  -> zz columns 0..63 = z_hx, 64..127 = z_hc
GATES = [("f", False), ("i", False), ("C", True), ("o", False)]

_COMPILED = {}


def _build(reps=1, ch=CH, wdt=None, order="chunk", split_dma=False):
    wdt = wdt or F32R
    nchunk = NKT // ch
    nc = bacc.Bacc(
        "TRN2", target_bir_lowering=False, debug=False, num_devices=NCORES
    )
    w_drams = [
        nc.dram_tensor(
            f"w_{g}", [nchunk, 128, ch * HS], wdt, kind="ExternalInput"
        ).ap()
        for g, _ in GATES
    ]
    zz_dram = nc.dram_tensor("zz", [128, 2 * NKT], F32, kind="ExternalInput").ap()
    par_dram = nc.dram_tensor("par", [1, 5 * HS], F32, kind="ExternalInput").ap()
    out_dram = nc.dram_tensor("out", [2, HS], F32, kind="ExternalOutput").ap()

    SIG = mybir.ActivationFunctionType.Sigmoid
    TANH = mybir.ActivationFunctionType.Tanh

    with tile.TileContext(nc) as tc:
        with (
            tc.tile_pool(name="const", bufs=1) as constp,
            tc.tile_pool(name="w", bufs=max(2, 64 // ch)) as wp,
            tc.tile_pool(name="tail", bufs=2) as tailp,
            tc.tile_pool(name="psum", bufs=2, space=bass.MemorySpace.PSUM) as psump,
        ):
            zz_f32 = constp.tile([128, 2 * NKT], F32, tag="zzf")
            nc.sync.dma_start(zz_f32[:], zz_dram[:])
            zz = constp.tile([128, 2 * NKT], wdt, tag="zz")
            nc.vector.tensor_copy(zz[:], zz_f32[:])
            par = constp.tile([1, 5 * HS], F32, tag="par")
            nc.sync.dma_start(par[:], par_dram[:])

            for rep in range(reps):
                psums = [
                    psump.tile([1, HS], F32, tag=f"ps{g}", name=f"ps{g}_{rep}")
                    for g, _ in GATES
                ]

                def w_dma(eng_idx):
                    if not split_dma:
                        return nc.sync
                    return nc.sync if eng_idx % 2 == 0 else nc.scalar

                def gate_matmuls(g_idx, c, wt):
                    _, use_hc = GATES[g_idx]
                    zbase = NKT if use_hc else 0
                    for t in range(ch):
                        kt = c * ch + t
                        nc.tensor.matmul(
                            psums[g_idx][0:1, :],
                            lhsT=zz[:, zbase + kt : zbase + kt + 1],
                            rhs=wt[:, t * HS : (t + 1) * HS],
                            start=(kt == 0),
                            stop=(kt == NKT - 1),
                        )

                acts = [None] * 4

                def gate_act(g_idx):
                    g = GATES[g_idx][0]
                    pre = tailp.tile([1, HS], F32, tag=f"pre{g}", name=f"pre{g}_{rep}")
                    nc.vector.tensor_add(
                        pre[:],
                        psums[g_idx][0:1, :],
                        par[0:1, g_idx * HS : (g_idx + 1) * HS],
                    )
                    act = tailp.tile([1, HS], F32, tag=f"act{g}", name=f"act{g}_{rep}")
                    nc.scalar.activation(act[:], pre[:], TANH if g == "C" else SIG)
                    acts[g_idx] = act

                if order == "chunk":
                    for c in range(nchunk):
                        wtiles = []
                        for g_idx, (g, _) in enumerate(GATES):
                            wt = wp.tile(
                                [128, ch * HS], wdt, tag="w", name=f"wt_{rep}_{c}_{g}"
                            )
                            w_dma(c * 4 + g_idx).dma_start(wt[:], w_drams[g_idx][c])
                            wtiles.append(wt)
                        for g_idx in range(4):
                            gate_matmuls(g_idx, c, wtiles[g_idx])
                    for g_idx in range(4):
                        gate_act(g_idx)
                else:  # gate-major: finish each gate fully, activation overlaps
                    for g_idx, (g, _) in enumerate(GATES):
                        for c in range(nchunk):
                            wt = wp.tile(
                                [128, ch * HS], wdt, tag="w", name=f"wt_{rep}_{c}_{g}"
                            )
                            w_dma(g_idx * nchunk + c).dma_start(
                                wt[:], w_drams[g_idx][c]
                            )
                            gate_matmuls(g_idx, c, wt)
                        gate_act(g_idx)

                f_t, i_t, ct_t, o_t = acts
                c_prev = par[0:1, 4 * HS : 5 * HS]
                m1 = tailp.tile([1, HS], F32, tag="m1", name=f"m1_{rep}")
                nc.vector.tensor_mul(m1[:], f_t[:], c_prev)
                m2 = tailp.tile([1, HS], F32, tag="m2", name=f"m2_{rep}")
                nc.vector.tensor_mul(m2[:], i_t[:], ct_t[:])
                c_new = tailp.tile([1, HS], F32, tag="cn", name=f"cn_{rep}")
                nc.vector.tensor_add(c_new[:], m1[:], m2[:])
                tc_t = tailp.tile([1, HS], F32, tag="tc", name=f"tc_{rep}")
                nc.scalar.activation(tc_t[:], c_new[:], TANH)
                h_new = tailp.tile([1, HS], F32, tag="hn", name=f"hn_{rep}")
                nc.vector.tensor_mul(h_new[:], o_t[:], tc_t[:])
                nc.sync.dma_start(out_dram[0:1, :], h_new[:])
                nc.sync.dma_start(out_dram[1:2, :], c_new[:])

    nc.compile()
    return nc


def _get_nc(reps=1, ch=CH, wdt=None, order="chunk", split_dma=False):
    key = (reps, ch, str(wdt), order, split_dma)
    if key not in _COMPILED:
        _COMPILED[key] = _build(reps, ch, wdt, order, split_dma)
    return _COMPILED[key]


def _prep_w(W, core, ch=CH, npdt=np.float32):
    """[H, 2H] gate weight -> core slice, transposed + chunk-tiled.

    Output [nchunk, 128, ch*HS] with out[c, p, t*HS + n] = W[core*HS + n,
    c*ch*128 + t*128 + p] so each chunk DMA is contiguous per partition.
    """
    nchunk = NKT // ch
    A = W[core * HS : (core + 1) * HS, :]          # [HS, K2]
    B = A.reshape(HS, nchunk, ch, 128)             # [n, c, t, p]
    out = np.ascontiguousarray(B.transpose(1, 3, 2, 0)).reshape(nchunk, 128, ch * HS)
    return out.astype(npdt) if npdt is not np.float32 else out


def _make_in_maps(inputs, ch=CH, npdt=np.float32):
    h = np.asarray(inputs["h_tmin1"], np.float32)
    c = np.asarray(inputs["c_tmin1"], np.float32)
    x = np.asarray(inputs["x_t"], np.float32)
    Ws = {
        "f": np.asarray(inputs["W_f"], np.float32),
        "i": np.asarray(inputs["W_i"], np.float32),
        "C": np.asarray(inputs["W_C"], np.float32),
        "o": np.asarray(inputs["W_o"], np.float32),
    }
    bs = {
        "f": np.asarray(inputs["b_f"], np.float32),
        "i": np.asarray(inputs["b_i"], np.float32),
        "C": np.asarray(inputs["b_C"], np.float32),
        "o": np.asarray(inputs["b_o"], np.float32),
    }

    z_hx = np.concatenate([h, x])                  # [2H]
    z_hc = np.concatenate([h, c])                  # [2H]
    # column kt of zz = k-tile kt of z (z_hx in 0..NKT, z_hc in NKT..2*NKT)
    zz = np.ascontiguousarray(
        np.concatenate(
            [z_hx.reshape(NKT, 128).T, z_hc.reshape(NKT, 128).T], axis=1
        )
    )

    in_maps = []
    for core in range(NCORES):
        sl = slice(core * HS, (core + 1) * HS)
        par = np.concatenate(
            [bs["f"][sl], bs["i"][sl], bs["C"][sl], bs["o"][sl], c[sl]]
        ).reshape(1, 5 * HS)
        m = {"zz": zz, "par": np.ascontiguousarray(par)}
        for g, _ in GATES:
            m[f"w_{g}"] = _prep_w(Ws[g], core, ch, npdt)
        in_maps.append(m)
    return in_maps


def kernel(**inputs):
    in_maps = _make_in_maps(inputs)
    nc = _get_nc()
    res = run_bass_kernel_spmd(nc, in_maps, list(range(NCORES)))
    outs = [res.results[i]["out"] for i in range(NCORES)]
    h_new = np.concatenate([o[0] for o in outs]).astype(np.float32)
    c_new = np.concatenate([o[1] for o in outs]).astype(np.float32)
    return (h_new, c_new)
